# revision 46
# baseline (speedup 1.0000x reference)
"""Self-contained MiniSpinNet kernel for 8 Trainium2 NeuronCores.

kernel(**inputs) takes the FULL unsharded inputs (as produced by
setup_inputs()) and returns the full (2, 64, 256) float32 descriptor.
Internally: data-parallel over the 128 B*M centers (16 per core), with
tiny cross-core AllReduces for the training-mode BatchNorm statistics.
"""
import numpy as np
import ml_dtypes

import concourse.bass as bass
import concourse.bacc as bacc
import concourse.mybir as mybir
import concourse.tile as tile
from concourse import bass_utils

F32 = mybir.dt.float32
BF16 = mybir.dt.bfloat16
AF = mybir.ActivationFunctionType
OP = mybir.AluOpType
AX = mybir.AxisListType

B, N, M = 2, 2048, 64
BM = B * M
S = 16
NCORES = 8
NCH = 16
EPS = 1e-5

COS_B = [float(np.float32(np.cos(j * np.pi / 8))) for j in range(1, 8)]
R2_B = [float(np.float32((j / 16.0) ** 2)) for j in range(1, 8)]

PD = 1000
PADW = S * PD + 160


def _build_nc(n_cores=NCORES, reps=1, extra_cc=0):
    nstat = 1.0 / (n_cores * S * 512)
    nc = bacc.Bacc("TRN2", target_bir_lowering=False, debug=False, num_devices=n_cores)

    pts_d = nc.dram_tensor("pts", [128, NCH, 3], F32, kind="ExternalInput")
    ctr_d = nc.dram_tensor("ctrv", [S * 3], F32, kind="ExternalInput")
    wr4_d = nc.dram_tensor("wr4", [3, 128], BF16, kind="ExternalInput")
    w1_d = nc.dram_tensor("w1p", [128, 9 * 128], BF16, kind="ExternalInput")
    w2_d = nc.dram_tensor("w2p", [128, 18 * 128], BF16, kind="ExternalInput")
    w3_d = nc.dram_tensor("w3p", [128, 54 * 128], BF16, kind="ExternalInput")
    gb_d = nc.dram_tensor("gb", [128, 36], F32, kind="ExternalInput")
    desc_d = nc.dram_tensor("desc", [S, 256], F32, kind="ExternalOutput")

    with tile.TileContext(nc) as tc:
        with (
            tc.tile_pool(name="wp", bufs=1) as wp,
            tc.tile_pool(name="bigp", bufs=1) as bigp,
            tc.tile_pool(name="maskp", bufs=1) as maskp,
            tc.tile_pool(name="sampp", bufs=4) as sampp,
            tc.tile_pool(name="drainp", bufs=2) as drainp,
            tc.tile_pool(name="statp", bufs=1) as statp,
            tc.tile_pool(name="psb", bufs=2, space="PSUM") as psb,
            tc.tile_pool(name="psg", bufs=6, space="PSUM") as psg,
            tc.tile_pool(name="dramp", bufs=1, space="DRAM") as dramp,
        ):
          for rep in range(reps):
                pts = wp.tile([128, NCH, 3], F32, tag="pts")
                if rep > 0:
                    # serialize reps for latency timing: scribble rep k's
                    # output into the pts tile, which the real pts load then
                    # overwrites (WAW) — rep k+1's compute chain hangs off pts
                    nc.sync.dma_start(pts[0:1, 0, :], desc_d.ap()[0:1, 0:3])
                ctrb = wp.tile([128, S, 3], F32, tag="ctrb")
                wr4 = wp.tile([3, 128], BF16, tag="wr4")
                w1p = wp.tile([128, 9 * 128], BF16, tag="w1p")
                w2p = wp.tile([128, 18 * 128], BF16, tag="w2p")
                w3p = wp.tile([128, 54 * 128], BF16, tag="w3p")
                gb = wp.tile([128, 36], F32, tag="gb")
                nc.sync.dma_start(pts[:], pts_d.ap())
                nc.sync.dma_start(
                    ctrb[:],
                    ctr_d.ap().rearrange("(s c) -> s c", s=S, c=3).unsqueeze(0).partition_broadcast(128))
                nc.sync.dma_start(wr4[:], wr4_d.ap())
                nc.sync.dma_start(w1p[:], w1_d.ap())
                nc.sync.dma_start(w2p[:], w2_d.ap())
                nc.sync.dma_start(w3p[:], w3_d.ap())
                nc.sync.dma_start(gb[:], gb_d.ap())

                pad1 = bigp.tile([128, PADW], BF16, tag="pad1")
                pad2 = bigp.tile([128, PADW], BF16, tag="pad2")
                pad3 = bigp.tile([128, PADW], BF16, tag="pad3")

                def box(pad, s, dd, dh, dw, p0=0, p1=128):
                    base = s * PD + dd * 100 + dh * 10 + dw
                    v = pad[p0:p1, base:base + 800]
                    v = v.rearrange("p (d x) -> p d x", d=8)[:, :, 0:80]
                    v = v.rearrange("p d (h y) -> p d h y", h=8)[:, :, :, 0:8]
                    return v

                def box2(pad, s0, dd, dh, dw, p0=0, p1=128, ns=2):
                    # ns-center-wide box view [p, ns, d, h, w]
                    base = s0 * PD + dd * 100 + dh * 10 + dw
                    v = pad[p0:p1, base:base + ns * PD]
                    v = v.rearrange("p (s x) -> p s x", s=ns)[:, :, 0:800]
                    v = v.rearrange("p s (d x) -> p s d x", d=8)[:, :, :, 0:80]
                    v = v.rearrange("p s d (h y) -> p s d h y", h=8)[:, :, :, :, 0:8]
                    return v

                # ---------------- binning ----------------
                # rel holds [x, y, z, 1] per point-center pair; the ones
                # column lets the whl product emit the count row for free.
                # coord-major: xs_/ys_/zs_ are contiguous rows, so the
                # rho2/compare chain runs at full DVE rate (interleaved
                # layout paid a ~4x strided-access penalty).
                rel = bigp.tile([128, 4, S, NCH], F32, tag="rel")
                nc.vector.tensor_tensor(
                    rel[:, 0:3],
                    pts[:].rearrange("p k c -> p c k").unsqueeze(2)
                    .broadcast_to([128, 3, S, NCH]),
                    ctrb[:].rearrange("p s c -> p c s").unsqueeze(3)
                    .broadcast_to([128, 3, S, NCH]),
                    op=OP.subtract)
                nc.vector.memset(rel[:, 3], 1.0)
                xs_ = rel[:, 0].rearrange("p s k -> p (s k)")
                ys_ = rel[:, 1].rearrange("p s k -> p (s k)")
                zs_ = rel[:, 2].rearrange("p s k -> p (s k)")

                SK = S * NCH
                rho2 = maskp.tile([128, SK], F32, tag="rho2")
                tmp = maskp.tile([128, SK], F32, tag="tmp")
                nc.vector.tensor_tensor(rho2[:], xs_, xs_, op=OP.mult)
                nc.vector.tensor_tensor(tmp[:], ys_, ys_, op=OP.mult)
                nc.vector.tensor_tensor(rho2[:], rho2[:], tmp[:], op=OP.add)
                nc.vector.tensor_tensor(tmp[:], zs_, zs_, op=OP.mult)
                nc.vector.tensor_tensor(rho2[:], rho2[:], tmp[:], op=OP.add)
                rhoe = maskp.tile([128, SK], F32, tag="rhoe")
                nc.scalar.activation(rhoe[:], rho2[:], AF.Sqrt)

                # radial one-hot chain on gpsimd (Pool), theta chain on DVE,
                # phi chain on gpsimd — splits the binning elementwise work
                # across the two vector-capable engines.
                thr = gb
                mm = maskp.tile([128, SK, 9], BF16, tag="mbuf_r")
                nc.vector.memset(mm[:, :, 0], 1.0)
                nc.vector.memset(mm[:, :, 8], 0.0)
                nc.vector.tensor_tensor(
                    mm[:, :, 1:8],
                    rho2[:].unsqueeze(2).broadcast_to([128, SK, 7]),
                    thr[:, 10:17].unsqueeze(1).broadcast_to([128, SK, 7]),
                    op=OP.is_ge)
                oh_r = maskp.tile([128, SK, 8], BF16, tag="oh_r")
                nc.vector.tensor_tensor(oh_r[:], mm[:, :, 0:8], mm[:, :, 1:9], op=OP.subtract)

                mt = maskp.tile([128, SK, 9], BF16, tag="mbuf")
                nc.vector.memset(mt[:, :, 0], 1.0)
                nc.vector.memset(mt[:, :, 8], 0.0)
                prodt = maskp.tile([128, SK, 7], F32, tag="prodt")
                nc.vector.tensor_tensor(
                    prodt[:],
                    rhoe[:].unsqueeze(2).broadcast_to([128, SK, 7]),
                    thr[:, 17:24].unsqueeze(1).broadcast_to([128, SK, 7]),
                    op=OP.mult)
                nc.vector.tensor_tensor(
                    mt[:, :, 1:8], prodt[:],
                    zs_.unsqueeze(2).broadcast_to([128, SK, 7]),
                    op=OP.is_gt)
                oh_t = maskp.tile([128, SK, 8], BF16, tag="oh_t")
                nc.vector.tensor_tensor(oh_t[:], mt[:, :, 0:8], mt[:, :, 1:9], op=OP.subtract)

                am = maskp.tile([128, SK], BF16, tag="am")
                bm = maskp.tile([128, SK], BF16, tag="bm")
                cm = maskp.tile([128, SK], BF16, tag="cm")
                ax_ = maskp.tile([128, SK], F32, tag="ax")
                ay_ = maskp.tile([128, SK], F32, tag="ay")
                nc.gpsimd.tensor_scalar(am[:], ys_, 0.0, None, op0=OP.is_ge)
                nc.gpsimd.tensor_scalar(bm[:], xs_, 0.0, None, op0=OP.is_ge)
                nc.scalar.activation(ax_[:], xs_, AF.Abs)
                nc.scalar.activation(ay_[:], ys_, AF.Abs)
                nc.vector.tensor_tensor(cm[:], ay_[:], ax_[:], op=OP.is_ge)
                na = maskp.tile([128, SK], BF16, tag="na")
                nb = maskp.tile([128, SK], BF16, tag="nb")
                ncc = maskp.tile([128, SK], BF16, tag="ncc")
                nc.gpsimd.tensor_scalar(na[:], am[:], -1.0, 1.0, op0=OP.mult, op1=OP.add)
                nc.gpsimd.tensor_scalar(nb[:], bm[:], -1.0, 1.0, op0=OP.mult, op1=OP.add)
                nc.gpsimd.tensor_scalar(ncc[:], cm[:], -1.0, 1.0, op0=OP.mult, op1=OP.add)
                t00 = maskp.tile([128, SK], BF16, tag="t00")
                t01 = maskp.tile([128, SK], BF16, tag="t01")
                t11 = maskp.tile([128, SK], BF16, tag="t11")
                t10 = maskp.tile([128, SK], BF16, tag="t10")
                nc.gpsimd.tensor_tensor(t00[:], na[:], nb[:], op=OP.mult)
                nc.gpsimd.tensor_tensor(t01[:], na[:], bm[:], op=OP.mult)
                nc.gpsimd.tensor_tensor(t11[:], am[:], bm[:], op=OP.mult)
                nc.gpsimd.tensor_tensor(t10[:], am[:], nb[:], op=OP.mult)
                oh_p = maskp.tile([128, SK, 8], BF16, tag="oh_p")
                for col, (t_, c_) in enumerate(
                        [(t00, ncc), (t00, cm), (t01, cm), (t01, ncc),
                         (t11, ncc), (t11, cm), (t10, cm), (t10, ncc)]):
                    nc.gpsimd.tensor_tensor(oh_p[:, :, col], t_[:], c_[:], op=OP.mult)

                sums = statp.tile([64, S, 4, 8], F32, tag="sums")
                for s in range(S):
                    sl = slice(s * NCH, (s + 1) * NCH)
                    eng = nc.vector if s < 10 else nc.gpsimd
                    ohrt = sampp.tile([128, NCH, 8, 8], BF16, tag="ohrt", name=f"ohrt{s}")
                    eng.tensor_tensor(
                        ohrt[:],
                        oh_r[:, sl, :].unsqueeze(3).broadcast_to([128, NCH, 8, 8]),
                        oh_t[:, sl, :].unsqueeze(2).broadcast_to([128, NCH, 8, 8]),
                        op=OP.mult)
                    whl = sampp.tile([128, NCH, 4, 8], BF16, tag="whl", name=f"whl{s}")
                    eng.tensor_tensor(
                        whl[:],
                        rel[:, :, s, :].rearrange("p c k -> p k c").unsqueeze(3)
                        .broadcast_to([128, NCH, 4, 8]),
                        oh_p[:, sl, :].unsqueeze(2).broadcast_to([128, NCH, 4, 8]),
                        op=OP.mult)

                    ps = psb.tile([64, 32], F32, tag="binp", name=f"binps{s}")
                    for k in range(NCH):
                        nc.tensor.matmul(ps[:], ohrt[:, k], whl[:, k],
                                         start=(k == 0), stop=(k == NCH - 1))
                    nc.scalar.activation(
                        sums[:, s, :, :].rearrange("p e f -> p (e f)"), ps[:],
                        AF.Copy)

                cnt = sums[:, :, 3, :]
                nc.vector.tensor_scalar(cnt, cnt, 1.0, None, op0=OP.max)
                nc.vector.reciprocal(cnt, cnt)
                binfb = statp.tile([64, S, 3, 8], BF16, tag="binfb")
                nc.vector.tensor_tensor(
                    binfb[:], sums[:, :, 0:3, :],
                    sums[:, :, 3:4, :].broadcast_to([64, S, 3, 8]), op=OP.mult)

                binf_d = dramp.tile([3, S * 512], BF16, tag="binfd")
                for c in range(3):
                    nc.sync.dma_start(
                        binf_d[c].rearrange("(s rt p) -> rt s p", s=S, rt=64, p=8),
                        binfb[:, :, c, :])

                # pad1 zero-fill split across Pool+DVE right after their
                # binning work; pad2/3 are emitted later, running while Pool
                # idles in conv phases
                zero1 = statp.tile([128, 1], BF16, tag="zero1")
                nc.vector.memset(zero1[:], 0.0)
                nc.gpsimd.memset(pad1[:, 0:PADW // 2], 0.0)

                epst = statp.tile([128, 1], F32, tag="epst")
                nc.vector.memset(epst[:], EPS)

                # ---------------- BN0 via binf moments ----------------
                # x = W_r @ binf is linear, so BN0 stats derive from binf's
                # first/second moments (12 numbers): the collective overlaps
                # the raise matmuls instead of stalling after them.
                mom = statp.tile([64, 9], F32, tag="mom")
                for c in range(3):
                    nc.vector.tensor_reduce(
                        mom[:, c:c + 1], binfb[:, :, c, :],
                        axis=AX.XY, op=OP.add)
                pairs = [(0, 0), (0, 1), (0, 2), (1, 1), (1, 2), (2, 2)]
                for j, (c1, c2) in enumerate(pairs):
                    mp = drainp.tile([64, S, 8], F32, tag="mprod", name=f"mp{j}")
                    nc.vector.tensor_tensor(mp[:], binfb[:, :, c1, :],
                                            binfb[:, :, c2, :], op=OP.mult)
                    nc.vector.tensor_reduce(
                        mom[:, 3 + j:4 + j],
                        mp[:].rearrange("p s f -> p (s f)"),
                        axis=AX.X, op=OP.add)
                ones64 = statp.tile([64, 1], F32, tag="ones64")
                nc.vector.memset(ones64[:], 1.0)
                mps = psb.tile([64, 32], F32, tag="binp", name="momp")
                nc.tensor.matmul(mps[0:1, 0:9], ones64[:], mom[:],
                                 start=True, stop=True)
                m9 = statp.tile([1, 9], F32, tag="m9")
                nc.scalar.activation(m9[:], mps[0:1, 0:9], AF.Copy)
                nc.scalar.activation(
                    pad1[:, PADW // 2:],
                    zero1[:].broadcast_to([128, PADW - PADW // 2]), AF.Copy)
                nc.scalar.activation(
                    pad2[:], zero1[:].broadcast_to([128, PADW]), AF.Copy)
                cci0 = dramp.tile([1, 9], F32, tag="cci0")
                cco0 = dramp.tile([n_cores, 9], F32, tag="cco0")
                # cci0 issues from the Act queue (right after m9) so it is
                # not stuck behind bulk SP transfers on the DMA device;
                # xr_all issues from the Pool queue after the collective, so
                # its 5us transfer runs during (not before) the collective.
                nc.scalar.dma_start(cci0[:], m9[:])
                if n_cores > 1:
                    nc.gpsimd.collective_compute(
                        "AllGather", OP.bypass,
                        replica_groups=[list(range(n_cores))],
                        ins=[cci0.opt()], outs=[cco0.opt()])
                else:
                    nc.sync.dma_start(cco0[:].rearrange("r x -> (r x)"),
                                      cci0[:].rearrange("p j -> (p j)"))
                xr_all = statp.tile([3, S * 512], BF16, tag="xrall")
                for q in range(4):
                    qs = q * 4 * 512
                    nc.sync.dma_start(xr_all[:, qs:qs + 4 * 512],
                                      binf_d[:, qs:qs + 4 * 512])
                gmb2 = statp.tile([128, n_cores, 9], F32, tag="gmb2")
                nc.sync.dma_start(
                    gmb2[:],
                    cco0[:].rearrange("r j -> (r j)").unsqueeze(0)
                    .partition_broadcast(128))
                gmb = statp.tile([128, 9], F32, tag="gmb")
                for j in range(9):
                    nc.vector.tensor_reduce(
                        gmb[:, j:j + 1], gmb2[:, :, j], axis=AX.X, op=OP.add)
                # mean = nstat * <Wrow, m>;  Ex2 = nstat * <qw, m2>
                mtmp = statp.tile([128, 9], F32, tag="mtmp")
                nc.vector.tensor_tensor(mtmp[:, 0:3], gmb[:, 0:3],
                                        gb[:, 24:27], op=OP.mult)
                nc.vector.tensor_tensor(mtmp[:, 3:9], gmb[:, 3:9],
                                        gb[:, 27:33], op=OP.mult)
                mean0 = statp.tile([128, 1], F32, tag="mean0")
                var0 = statp.tile([128, 1], F32, tag="var0")
                sc0 = statp.tile([128, 1], F32, tag="sc0")
                bi0 = statp.tile([128, 1], F32, tag="bi0")
                nc.vector.tensor_reduce(mean0[:], mtmp[:, 0:3], axis=AX.X, op=OP.add)
                nc.vector.tensor_scalar(mean0[:], mean0[:], nstat, None, op0=OP.mult)
                nc.vector.tensor_reduce(var0[:], mtmp[:, 3:9], axis=AX.X, op=OP.add)
                nc.vector.tensor_scalar(var0[:], var0[:], nstat, None, op0=OP.mult)
                nc.vector.tensor_tensor(mtmp[:, 0:1], mean0[:], mean0[:], op=OP.mult)
                nc.vector.tensor_tensor(var0[:], var0[:], mtmp[:, 0:1], op=OP.subtract)
                nc.scalar.activation(var0[:], var0[:], AF.Sqrt, bias=epst[:])
                nc.vector.reciprocal(var0[:], var0[:])
                nc.vector.tensor_tensor(sc0[:], gb[:, 0:1], var0[:], op=OP.mult)
                nc.vector.tensor_tensor(bi0[:], mean0[:], sc0[:], op=OP.mult)
                nc.vector.tensor_tensor(bi0[:], gb[:, 1:2], bi0[:], op=OP.subtract)

                # ---------------- raising (BN0+ReLU fused in drain) --------
                # The 32 raise channels are written as 3 w-shifted copies in
                # pad1's partition groups (copy g interior at w-offset g), so
                # conv1 can contract 3 taps per matmul (9 reads vs 27).
                CH = S // 4
                for c0 in range(0, S, CH):
                    for s in range(c0, c0 + CH):
                        xps = psg.tile([128, 512], F32, tag="big", name=f"xps{s}")
                        nc.tensor.matmul(
                            xps[:], wr4[:],
                            xr_all[:, s * 512:(s + 1) * 512],
                            start=True, stop=True)
                        v = box(pad1, s, 1, 1, 1, p0=32, p1=64)
                        xv = xps[32:64].rearrange("p (d h w) -> p d h w", d=8, h=8)
                        if s < 11:
                            nc.scalar.activation(v, xv, AF.Relu,
                                                 bias=bi0[32:64], scale=sc0[32:64])
                        else:
                            t0 = drainp.tile([32, 8, 8, 8], BF16, tag="rl0",
                                             name=f"rl0_{s}")
                            nc.vector.tensor_scalar(t0[:], xv, sc0[32:64],
                                                    bi0[32:64],
                                                    op0=OP.mult, op1=OP.add)
                            nc.vector.tensor_scalar(v, t0[:], 0.0, None, op0=OP.max)
                    a, b = c0 * PD, (c0 + CH) * PD
                    nc.sync.dma_start(pad1[0:32, a:b - 1], pad1[32:64, a + 1:b])
                    nc.sync.dma_start(pad1[64:96, a + 1:b], pad1[32:64, a:b - 1])

                def bn_allreduce(st, layer, dup64=False):
                    # AllGather + local reduce: collective latency is ~1.9x
                    # smaller than AllReduce for these tiny payloads.
                    ncols = st.shape[-1]
                    cc_in = dramp.tile([128, ncols], F32, tag=f"cci{layer}")
                    cc_out = dramp.tile([n_cores, 128 * ncols], F32, tag=f"cco{layer}")
                    nc.sync.dma_start(cc_in[:], st[:])
                    if n_cores > 1:
                        nc.gpsimd.collective_compute(
                            "AllGather", OP.bypass,
                            replica_groups=[list(range(n_cores))],
                            ins=[cc_in.opt()], outs=[cc_out.opt()])
                    else:
                        nc.sync.dma_start(
                            cc_out[:].rearrange("r x -> (r x)"),
                            cc_in[:].rearrange("p j -> (p j)"))
                    grt = statp.tile([128, ncols, n_cores], F32, tag=f"grt{layer}")
                    src_ap = cc_out[:].rearrange("r (p j) -> p j r", p=128, j=ncols)
                    if dup64:
                        nc.sync.dma_start(grt[0:64], src_ap[0:64])
                        nc.sync.dma_start(grt[64:128], src_ap[0:64])
                    else:
                        nc.sync.dma_start(grt[:], src_ap)
                    gst = statp.tile([128, ncols], F32, tag=f"gst{layer}")
                    for j in range(ncols):
                        nc.vector.tensor_reduce(
                            gst[:, j:j + 1], grt[:, j, :], axis=AX.X, op=OP.add)
                    return gst

                def bn_math(gp, gq, gcol, bcol, layer, off=0):
                    mean = statp.tile([128, 1], F32, tag=f"mean{layer}_{off}")
                    var = statp.tile([128, 1], F32, tag=f"var{layer}_{off}")
                    sc = statp.tile([128, 1], F32, tag=f"sc{layer}_{off}")
                    bi = statp.tile([128, 1], F32, tag=f"bi{layer}_{off}")
                    nc.vector.tensor_scalar(mean[:], gp, nstat, None, op0=OP.mult)
                    nc.vector.tensor_tensor(var[:], mean[:], mean[:], op=OP.mult)
                    nc.vector.scalar_tensor_tensor(var[:], gq, nstat, var[:],
                                                   op0=OP.mult, op1=OP.subtract)
                    nc.scalar.activation(var[:], var[:], AF.Sqrt, bias=epst[:])
                    nc.vector.reciprocal(var[:], var[:])
                    nc.vector.tensor_tensor(sc[:], gb[:, gcol:gcol + 1], var[:], op=OP.mult)
                    nc.vector.tensor_tensor(bi[:], mean[:], sc[:], op=OP.mult)
                    nc.vector.tensor_tensor(bi[:], gb[:, bcol:bcol + 1], bi[:], op=OP.subtract)
                    return sc, bi

                def bn_block(ptile, qtile, gcol, bcol, layer, dup64=False):
                    st = statp.tile([128, 2], F32, tag=f"st{layer}")
                    nc.vector.tensor_reduce(st[:, 0:1], ptile[:], axis=AX.X, op=OP.add)
                    nc.vector.tensor_reduce(st[:, 1:2], qtile[:], axis=AX.X, op=OP.add)
                    gst = bn_allreduce(st, layer, dup64)
                    return bn_math(gst[:, 0:1], gst[:, 1:2], gcol, bcol, layer)

                # conv1: 9 reads/sample, each contracting 3 w-taps across the
                # shifted pad1 copies. Output 64ch written as 2 w-shifted
                # copies in pad2 so conv2 can contract 2 taps per matmul.
                pt1 = statp.tile([128, S], F32, tag="pt1")
                qt1 = statp.tile([128, S], F32, tag="qt1")
                nc.vector.memset(pt1[:], 0.0)
                nc.vector.memset(qt1[:], 0.0)
                for s0 in range(0, S, 4):
                    pss = [psg.tile([128, 512], F32, tag="big", name=f"c1ps_{s0}_{g}")
                           for g in range(4)]
                    for r in range(9):
                        a, bb = r // 3, r % 3
                        for g in range(4):
                            nc.tensor.matmul(pss[g][:], w1p[:, r * 128:(r + 1) * 128],
                                             box(pad1, s0 + g, a, bb, 1),
                                             start=(r == 0), stop=(r == 8))
                    for g in range(4):
                        s = s0 + g
                        v = box(pad2, s, 1, 1, 0, p0=0, p1=64)
                        nc.scalar.activation(
                            v, pss[g][0:64].rearrange("p (d h w) -> p d h w", d=8, h=8),
                            AF.Copy, accum_out=pt1[0:64, s:s + 1])
                        sq = drainp.tile([64, 8, 8, 8], BF16, tag="sqc1", name=f"sqc1_{s0}_{g}")
                        nc.vector.tensor_tensor(sq[:], v, v, op=OP.mult)
                        nc.vector.tensor_reduce(
                            qt1[0:64, s:s + 1],
                            sq[:].rearrange("p d h w -> p (d h w)"),
                            axis=AX.X, op=OP.add)
                    # replicate the raw group into copy 1 now (overlaps conv1
                    # and the BN1 collective); relu is applied per copy later.
                    a, b = s0 * PD, (s0 + 4) * PD
                    nc.sync.dma_start(pad2[64:128, a + 1:b], pad2[0:64, a:b - 1])
                sc1, bi1 = bn_block(pt1, qt1, 2, 3, 1, dup64=True)
                for s in range(S):
                    v = box(pad2, s, 1, 1, 0, p0=0, p1=64)
                    nc.scalar.activation(v, v, AF.Relu,
                                         bias=bi1[0:64], scale=sc1[0:64])
                    v2 = box(pad2, s, 1, 1, 1, p0=64, p1=128)
                    if s % 4 == 3:
                        nc.scalar.activation(v2, v2, AF.Relu,
                                             bias=bi1[64:128], scale=sc1[64:128])
                    else:
                        t2 = drainp.tile([64, 8, 8, 8], BF16, tag="rl1", name=f"rl1_{s}")
                        nc.vector.tensor_scalar(t2[:], v2, sc1[64:128], bi1[64:128],
                                                op0=OP.mult, op1=OP.add)
                        nc.vector.tensor_scalar(v2, t2[:], 0.0, None, op0=OP.max)

                # conv2: 18 reads/sample (2 per (d,h) tap pair), contracting
                # 2 w-taps across the shifted pad2 copies.
                nc.gpsimd.memset(pad3[:], 0.0)
                pt2 = statp.tile([128, S], F32, tag="pt2")
                qt2 = statp.tile([128, S], F32, tag="qt2")
                for s0 in range(0, S, 4):
                    pss = [psg.tile([128, 512], F32, tag="big", name=f"c2ps_{s0}_{g}")
                           for g in range(4)]
                    for r in range(18):
                        a, bb, tw = r // 6, (r // 2) % 3, r % 2
                        for g in range(4):
                            nc.tensor.matmul(pss[g][:], w2p[:, r * 128:(r + 1) * 128],
                                             box(pad2, s0 + g, a, bb, tw),
                                             start=(r == 0), stop=(r == 17))
                    for g in range(4):
                        s = s0 + g
                        v = box(pad3, s, 1, 1, 1)
                        nc.scalar.activation(
                            v, pss[g][:].rearrange("p (d h w) -> p d h w", d=8, h=8),
                            AF.Copy, accum_out=pt2[:, s:s + 1])
                        sq = drainp.tile([128, 8, 8, 8], BF16, tag="sq", name=f"sqc2_{s0}_{g}")
                        nc.vector.tensor_tensor(sq[:], v, v, op=OP.mult)
                        nc.vector.tensor_reduce(
                            qt2[:, s:s + 1],
                            sq[:].rearrange("p d h w -> p (d h w)"),
                            axis=AX.X, op=OP.add)
                sc2, bi2 = bn_block(pt2, qt2, 4, 5, 2)
                for s in range(S):
                    v = box(pad3, s, 1, 1, 1)
                    if s % 2 == 0:
                        nc.scalar.activation(v, v, AF.Relu, bias=bi2[:], scale=sc2[:])
                    else:
                        t2 = drainp.tile([128, 8, 8, 8], BF16, tag="rl2", name=f"rl2_{s}")
                        nc.vector.tensor_scalar(t2[:], v, sc2[:], bi2[:],
                                                op0=OP.mult, op1=OP.add)
                        nc.vector.tensor_scalar(v, t2[:], 0.0, None, op0=OP.max)

                # h-outer: half 0's BN3 collective overlaps half 1's convs
                desc_sb = statp.tile([128, 2, S], F32, tag="descsb")
                for h, (gcol, bcol) in ((0, (6, 7)), (1, (8, 9))):
                    pt3 = statp.tile([128, S], F32, tag=f"pt3_{h}")
                    qt3 = statp.tile([128, S], F32, tag=f"qt3_{h}")
                    mxs = statp.tile([128, S], F32, tag=f"mxs_{h}")
                    mns = statp.tile([128, S], F32, tag=f"mns_{h}")
                    for s in range(S):
                        ps = psg.tile([128, 512], F32, tag="big", name=f"c3ps_{s}_{h}")
                        for o in range(27):
                            dd, dh, dw = o // 9, (o // 3) % 3, o % 3
                            lhsT = w3p[:, (o * 2 + h) * 128:(o * 2 + h + 1) * 128]
                            rhs = box(pad3, s, dd, dh, dw)
                            nc.tensor.matmul(ps[:], lhsT, rhs, start=(o == 0), stop=(o == 26))
                        nc.vector.tensor_reduce(mxs[:, s:s + 1], ps[:], axis=AX.X, op=OP.max)
                        nc.vector.tensor_reduce(mns[:, s:s + 1], ps[:], axis=AX.X, op=OP.min)
                        nc.vector.tensor_reduce(pt3[:, s:s + 1], ps[:], axis=AX.X, op=OP.add)
                        sq = drainp.tile([128, 512], BF16, tag="sqf", name=f"sqf_{s}_{h}")
                        nc.scalar.activation(sq[:], ps[:], AF.Square)
                        nc.vector.tensor_reduce(qt3[:, s:s + 1], sq[:], axis=AX.X, op=OP.add)

                    sc3, bi3 = bn_block(pt3, qt3, gcol, bcol, 3 + h)
                    zmx = statp.tile([128, S], F32, tag=f"zmx{h}")
                    zmn = statp.tile([128, S], F32, tag=f"zmn{h}")
                    nc.vector.tensor_scalar(zmx[:], mxs[:], sc3[:], bi3[:],
                                            op0=OP.mult, op1=OP.add)
                    nc.vector.tensor_scalar(zmn[:], mns[:], sc3[:], bi3[:],
                                            op0=OP.mult, op1=OP.add)
                    csel = statp.tile([128, 1], F32, tag=f"csel{h}")
                    nc.vector.tensor_scalar(csel[:], sc3[:], 0.0, None, op0=OP.is_ge)
                    nc.vector.tensor_tensor(zmx[:], zmx[:], zmn[:], op=OP.subtract)
                    nc.vector.scalar_tensor_tensor(zmx[:], zmx[:], csel[:], zmn[:],
                                                   op0=OP.mult, op1=OP.add)
                    nc.vector.tensor_scalar(desc_sb[:, h, :], zmx[:], 0.0, None, op0=OP.max)
                    nc.sync.dma_start(
                        desc_d.ap().rearrange("s (hh ch) -> ch hh s", hh=2)[:, h, :],
                        desc_sb[:, h, :])

                if extra_cc:
                    # calibration chain: extra_cc serial AllGathers, each
                    # reading row 0 of the previous gather's output
                    xa = dramp.tile([n_cores, 16], F32, tag="xcc_a")
                    xb = dramp.tile([n_cores, 16], F32, tag="xcc_b")
                    nc.sync.dma_start(
                        xa[:].rearrange("r x -> (r x)")[0:16],
                        desc_sb[0:1, 0, 0:16].rearrange("p a -> (p a)"))
                    bufs_cc = [xa, xb]
                    for i in range(extra_cc):
                        src_t, dst_t = bufs_cc[i % 2], bufs_cc[(i + 1) % 2]
                        nc.gpsimd.collective_compute(
                            "AllGather", OP.bypass,
                            replica_groups=[list(range(n_cores))],
                            ins=[src_t[0:1, :]], outs=[dst_t.opt()])

    nc.compile()
    return nc


def _host_pack(inputs):
    pts_all = np.asarray(inputs["points"], np.float32)
    ctr_all = np.asarray(inputs["center_points"], np.float32)
    w_raise = np.asarray(inputs["w_raise"], np.float32)
    w1 = np.asarray(inputs["w1"], np.float32)
    w2 = np.asarray(inputs["w2"], np.float32)
    w3 = np.asarray(inputs["w3"], np.float32)

    wr4 = np.zeros((3, 128), np.float32)
    for g in range(4):
        wr4[:, g * 32:(g + 1) * 32] = w_raise.T
    # conv1: 9 reads at (a, b, 1); pad1 copy g (partitions 32g:32g+32) is
    # written at w-offset g, so at read (a, b, 1) it contributes tap
    # (a, b, 2-g). Copy 3 (partitions 96:128) is unused (zero weights/data).
    w1f = w1.reshape(64, 32, 27)
    w1p = np.zeros((128, 9 * 128), np.float32)
    for a in range(3):
        for bb in range(3):
            r = a * 3 + bb
            for g in range(3):
                o = a * 9 + bb * 3 + (2 - g)
                blk = np.concatenate([w1f[:, :, o].T, w1f[:, :, o].T], axis=1)
                w1p[32 * g:32 * (g + 1), r * 128:(r + 1) * 128] = blk
    # conv2: 2 reads per (a, b): read (a, b, 0) gives taps w=1 (copy 0,
    # written at w-offset 0) and w=0 (copy 1, at w-offset 1); read (a, b, 1)
    # gives tap w=2 via copy 0 (copy 1 rows zeroed to avoid double-count).
    w2f = w2.reshape(128, 64, 27)
    w2p = np.zeros((128, 18 * 128), np.float32)
    for a in range(3):
        for bb in range(3):
            r2 = (a * 3 + bb) * 2
            w2p[0:64, r2 * 128:(r2 + 1) * 128] = w2f[:, :, a * 9 + bb * 3 + 1].T
            w2p[64:128, r2 * 128:(r2 + 1) * 128] = w2f[:, :, a * 9 + bb * 3 + 0].T
            w2p[0:64, (r2 + 1) * 128:(r2 + 2) * 128] = w2f[:, :, a * 9 + bb * 3 + 2].T
    w3f = w3.reshape(256, 128, 27)
    w3p = np.zeros((128, 54 * 128), np.float32)
    for o in range(27):
        for h in range(2):
            w3p[:, (o * 2 + h) * 128:(o * 2 + h + 1) * 128] = \
                w3f[h * 128:(h + 1) * 128, :, o].T
    gb = np.zeros((128, 36), np.float32)
    g0 = np.asarray(inputs["g0"], np.float32); be0 = np.asarray(inputs["be0"], np.float32)
    g1 = np.asarray(inputs["g1"], np.float32); be1 = np.asarray(inputs["be1"], np.float32)
    g2 = np.asarray(inputs["g2"], np.float32); be2 = np.asarray(inputs["be2"], np.float32)
    g3 = np.asarray(inputs["g3"], np.float32); be3 = np.asarray(inputs["be3"], np.float32)
    gb[:, 0] = np.tile(g0, 4); gb[:, 1] = np.tile(be0, 4)
    gb[:, 2] = np.tile(g1, 2); gb[:, 3] = np.tile(be1, 2)
    gb[:, 4] = g2; gb[:, 5] = be2
    gb[:, 6] = g3[:128]; gb[:, 7] = be3[:128]
    gb[:, 8] = g3[128:]; gb[:, 9] = be3[128:]
    gb[:, 10:17] = np.asarray(R2_B, np.float32)[None, :]
    gb[:, 17:24] = np.asarray(COS_B, np.float32)[None, :]
    wr_rows = np.tile(w_raise, (4, 1))                    # [128, 3]
    gb[:, 24:27] = wr_rows
    qw = np.stack([wr_rows[:, 0] ** 2, 2 * wr_rows[:, 0] * wr_rows[:, 1],
                   2 * wr_rows[:, 0] * wr_rows[:, 2], wr_rows[:, 1] ** 2,
                   2 * wr_rows[:, 1] * wr_rows[:, 2], wr_rows[:, 2] ** 2], axis=1)
    gb[:, 27:33] = qw                                     # [128, 6]

    shared = dict(
        wr4=wr4.astype(ml_dtypes.bfloat16),
        w1p=w1p.astype(ml_dtypes.bfloat16),
        w2p=w2p.astype(ml_dtypes.bfloat16),
        w3p=w3p.astype(ml_dtypes.bfloat16),
        gb=gb,
    )
    in_maps = []
    for c in range(NCORES):
        b = c // 4
        pts = np.ascontiguousarray(np.transpose(pts_all[b].reshape(NCH, 128, 3), (1, 0, 2)))
        s0 = (c % 4) * S
        ctrv = np.ascontiguousarray(ctr_all[b, s0:s0 + S, :].reshape(-1))
        in_maps.append(dict(pts=pts, ctrv=ctrv, **shared))
    return in_maps


_CACHED_NC = None


def _get_nc():
    global _CACHED_NC
    if _CACHED_NC is None:
        _CACHED_NC = _build_nc(NCORES)
    return _CACHED_NC


class _Runner:
    """Persistent SPMD executor: jit once, keep stable inputs device-resident.

    Mirrors bass2jax.run_bass_via_pjrt's axon path, but the jitted
    executable and the (large, call-invariant) weight buffers live across
    calls, so each call ships only the small per-call tensors.
    """

    def __init__(self, nc, n_cores, static_names=(), chain=1):
        import jax
        import concourse.mybir as _mybir
        from jax.sharding import Mesh, PartitionSpec, NamedSharding
        from jax.experimental.shard_map import shard_map
        from concourse.bass2jax import (
            _bass_exec_p, partition_id_tensor, install_neuronx_cc_hook)

        install_neuronx_cc_hook()
        self.jax = jax
        self.nc = nc
        self.n_cores = n_cores
        self.static_names = set(static_names)
        self._static_cache = {}

        pname = nc.partition_id_tensor.name if nc.partition_id_tensor else None
        in_names, out_names, out_avals, zero_shapes = [], [], [], []
        for alloc in nc.m.functions[0].allocations:
            if not isinstance(alloc, _mybir.MemoryLocationSet):
                continue
            name = alloc.memorylocations[0].name
            if alloc.kind == "ExternalInput":
                if name != pname:
                    in_names.append(name)
            elif alloc.kind == "ExternalOutput":
                out_names.append(name)
                shape = tuple(alloc.tensor_shape)
                dtype = _mybir.dt.np(alloc.dtype)
                out_avals.append(jax.core.ShapedArray(shape, dtype))
                zero_shapes.append((shape, dtype))
        self.in_names, self.out_names = in_names, out_names
        self.out_avals, self.zero_shapes = out_avals, zero_shapes
        n_params, n_outs = len(in_names), len(out_names)
        in_names_full = in_names + out_names + ([pname] if pname else [])

        def _body(*args):
            ins, zeros = list(args[:n_params]), list(args[n_params:])
            pid = [partition_id_tensor()] if pname is not None else []
            for _ in range(chain):
                zeros = list(_bass_exec_p.bind(
                    *ins, *zeros, *pid, out_avals=tuple(out_avals),
                    in_names=tuple(in_names_full), out_names=tuple(out_names),
                    lowering_input_output_aliases=(), sim_require_finite=True,
                    sim_require_nnan=True, nc=nc))
            return tuple(zeros)

        devices = jax.devices()[:n_cores]
        self.mesh = Mesh(np.array(devices), ("core",))
        self.sharding = NamedSharding(self.mesh, PartitionSpec("core"))
        in_specs = (PartitionSpec("core"),) * (n_params + n_outs)
        out_specs = (PartitionSpec("core"),) * n_outs
        donate = tuple(range(n_params, n_params + n_outs))
        self.sharded = jax.jit(
            shard_map(_body, mesh=self.mesh, in_specs=in_specs,
                      out_specs=out_specs, check_rep=False),
            donate_argnums=donate, keep_unused=True)

    def __call__(self, in_maps):
        jax = self.jax
        args = []
        for i, name in enumerate(self.in_names):
            if name in self.static_names and name in self._static_cache:
                args.append(self._static_cache[name])
                continue
            cat = np.concatenate(
                [np.asarray(in_maps[c][name]) for c in range(self.n_cores)],
                axis=0)
            arr = jax.device_put(cat, self.sharding)
            if name in self.static_names:
                self._static_cache[name] = arr
            args.append(arr)
        for shape, dtype in self.zero_shapes:
            args.append(np.zeros((self.n_cores * shape[0], *shape[1:]), dtype))
        outs = self.sharded(*args)
        return [
            {name: np.asarray(outs[i]).reshape(self.n_cores, *self.out_avals[i].shape)[c]
             for i, name in enumerate(self.out_names)}
            for c in range(self.n_cores)
        ]


_RUNNER = None
_WKEY = None

_STATIC_INPUTS = ("wr4", "w1p", "w2p", "w3p", "gb")


def _weights_key(in_maps):
    import hashlib
    h = hashlib.sha1()
    for name in _STATIC_INPUTS:
        h.update(np.ascontiguousarray(in_maps[0][name]).tobytes())
    return h.digest()


def kernel(**inputs) -> np.ndarray:
    global _RUNNER, _WKEY
    nc = _get_nc()
    in_maps = _host_pack(inputs)
    try:
        if _RUNNER is None:
            _RUNNER = _Runner(nc, NCORES, static_names=_STATIC_INPUTS)
        wkey = _weights_key(in_maps)
        if wkey != _WKEY:
            _RUNNER._static_cache.clear()
            _WKEY = wkey
        results = _RUNNER(in_maps)
    except Exception:
        res = bass_utils.run_bass_kernel_spmd(
            nc, in_maps, core_ids=list(range(NCORES)))
        results = res.results
    out = np.concatenate([np.asarray(results[c]["desc"], np.float32)
                          for c in range(NCORES)], axis=0)     # (128, 256)
    return out.reshape(B, M, 256)



# revision 48
# speedup vs baseline: 1.2824x; 1.2824x over previous
"""Self-contained MiniSpinNet kernel for 8 Trainium2 NeuronCores.

kernel(**inputs) takes the FULL unsharded inputs (as produced by
setup_inputs()) and returns the full (2, 64, 256) float32 descriptor.
Internally: data-parallel over the 128 B*M centers (16 per core), with
tiny cross-core AllReduces for the training-mode BatchNorm statistics.
"""
import numpy as np
import ml_dtypes

import concourse.bass as bass
import concourse.bacc as bacc
import concourse.mybir as mybir
import concourse.tile as tile
from concourse import bass_utils

F32 = mybir.dt.float32
BF16 = mybir.dt.bfloat16
AF = mybir.ActivationFunctionType
OP = mybir.AluOpType
AX = mybir.AxisListType

B, N, M = 2, 2048, 64
BM = B * M
S = 16
NCORES = 8
NCH = 16
EPS = 1e-5

COS_B = [float(np.float32(np.cos(j * np.pi / 8))) for j in range(1, 8)]
R2_B = [float(np.float32((j / 16.0) ** 2)) for j in range(1, 8)]

PD = 1000
PADW = S * PD + 160


def _build_nc(n_cores=NCORES, reps=1, extra_cc=0):
    nstat = 1.0 / (n_cores * S * 512)
    nc = bacc.Bacc("TRN2", target_bir_lowering=False, debug=False, num_devices=n_cores)

    pts_d = nc.dram_tensor("pts", [128, NCH, 3], F32, kind="ExternalInput")
    ctr_d = nc.dram_tensor("ctrv", [S * 3], F32, kind="ExternalInput")
    wr4_d = nc.dram_tensor("wr4", [3, 128], BF16, kind="ExternalInput")
    w1_d = nc.dram_tensor("w1p", [128, 9 * 128], BF16, kind="ExternalInput")
    w2_d = nc.dram_tensor("w2p", [128, 18 * 128], BF16, kind="ExternalInput")
    w3_d = nc.dram_tensor("w3p", [128, 54 * 128], BF16, kind="ExternalInput")
    gb_d = nc.dram_tensor("gb", [128, 36], F32, kind="ExternalInput")
    desc_d = nc.dram_tensor("desc", [S, 256], F32, kind="ExternalOutput")

    with tile.TileContext(nc) as tc:
        with (
            tc.tile_pool(name="wp", bufs=1) as wp,
            tc.tile_pool(name="bigp", bufs=1) as bigp,
            tc.tile_pool(name="maskp", bufs=1) as maskp,
            tc.tile_pool(name="sampp", bufs=4) as sampp,
            tc.tile_pool(name="drainp", bufs=2) as drainp,
            tc.tile_pool(name="statp", bufs=1) as statp,
            tc.tile_pool(name="psb", bufs=2, space="PSUM") as psb,
            tc.tile_pool(name="psg", bufs=6, space="PSUM") as psg,
            tc.tile_pool(name="dramp", bufs=1, space="DRAM") as dramp,
        ):
          for rep in range(reps):
                pts = wp.tile([128, NCH, 3], F32, tag="pts")
                if rep > 0:
                    # serialize reps for latency timing: scribble rep k's
                    # output into the pts tile, which the real pts load then
                    # overwrites (WAW) — rep k+1's compute chain hangs off pts
                    nc.sync.dma_start(pts[0:1, 0, :], desc_d.ap()[0:1, 0:3])
                ctrb = wp.tile([128, S, 3], F32, tag="ctrb")
                wr4 = wp.tile([3, 128], BF16, tag="wr4")
                w1p = wp.tile([128, 9 * 128], BF16, tag="w1p")
                w2p = wp.tile([128, 18 * 128], BF16, tag="w2p")
                w3p = wp.tile([128, 54 * 128], BF16, tag="w3p")
                gb = wp.tile([128, 36], F32, tag="gb")
                nc.sync.dma_start(pts[:], pts_d.ap())
                nc.sync.dma_start(
                    ctrb[:],
                    ctr_d.ap().rearrange("(s c) -> s c", s=S, c=3).unsqueeze(0).partition_broadcast(128))
                nc.sync.dma_start(wr4[:], wr4_d.ap())
                nc.sync.dma_start(w1p[:], w1_d.ap())
                nc.sync.dma_start(w2p[:], w2_d.ap())
                nc.sync.dma_start(w3p[:], w3_d.ap())
                nc.sync.dma_start(gb[:], gb_d.ap())

                pad1 = bigp.tile([128, PADW], BF16, tag="pad1")
                pad2 = bigp.tile([128, PADW], BF16, tag="pad2")
                pad3 = bigp.tile([128, PADW], BF16, tag="pad3")

                def box(pad, s, dd, dh, dw, p0=0, p1=128):
                    base = s * PD + dd * 100 + dh * 10 + dw
                    v = pad[p0:p1, base:base + 800]
                    v = v.rearrange("p (d x) -> p d x", d=8)[:, :, 0:80]
                    v = v.rearrange("p d (h y) -> p d h y", h=8)[:, :, :, 0:8]
                    return v

                def box2(pad, s0, dd, dh, dw, p0=0, p1=128, ns=2):
                    # ns-center-wide box view [p, ns, d, h, w]
                    base = s0 * PD + dd * 100 + dh * 10 + dw
                    v = pad[p0:p1, base:base + ns * PD]
                    v = v.rearrange("p (s x) -> p s x", s=ns)[:, :, 0:800]
                    v = v.rearrange("p s (d x) -> p s d x", d=8)[:, :, :, 0:80]
                    v = v.rearrange("p s d (h y) -> p s d h y", h=8)[:, :, :, :, 0:8]
                    return v

                # ---------------- binning ----------------
                # rel holds [x, y, z, 1] per point-center pair; the ones
                # column lets the whl product emit the count row for free.
                # coord-major: xs_/ys_/zs_ are contiguous rows, so the
                # rho2/compare chain runs at full DVE rate (interleaved
                # layout paid a ~4x strided-access penalty).
                rel = bigp.tile([128, 4, S, NCH], F32, tag="rel")
                nc.vector.tensor_tensor(
                    rel[:, 0:3],
                    pts[:].rearrange("p k c -> p c k").unsqueeze(2)
                    .broadcast_to([128, 3, S, NCH]),
                    ctrb[:].rearrange("p s c -> p c s").unsqueeze(3)
                    .broadcast_to([128, 3, S, NCH]),
                    op=OP.subtract)
                nc.vector.memset(rel[:, 3], 1.0)
                xs_ = rel[:, 0].rearrange("p s k -> p (s k)")
                ys_ = rel[:, 1].rearrange("p s k -> p (s k)")
                zs_ = rel[:, 2].rearrange("p s k -> p (s k)")

                SK = S * NCH
                rho2 = maskp.tile([128, SK], F32, tag="rho2")
                tmp = maskp.tile([128, SK], F32, tag="tmp")
                nc.vector.tensor_tensor(rho2[:], xs_, xs_, op=OP.mult)
                nc.vector.tensor_tensor(tmp[:], ys_, ys_, op=OP.mult)
                nc.vector.tensor_tensor(rho2[:], rho2[:], tmp[:], op=OP.add)
                nc.vector.tensor_tensor(tmp[:], zs_, zs_, op=OP.mult)
                nc.vector.tensor_tensor(rho2[:], rho2[:], tmp[:], op=OP.add)
                rhoe = maskp.tile([128, SK], F32, tag="rhoe")
                nc.scalar.activation(rhoe[:], rho2[:], AF.Sqrt)

                # radial one-hot chain on gpsimd (Pool), theta chain on DVE,
                # phi chain on gpsimd — splits the binning elementwise work
                # across the two vector-capable engines.
                thr = gb
                mm = maskp.tile([128, SK, 9], BF16, tag="mbuf_r")
                nc.vector.memset(mm[:, :, 0], 1.0)
                nc.vector.memset(mm[:, :, 8], 0.0)
                nc.vector.tensor_tensor(
                    mm[:, :, 1:8],
                    rho2[:].unsqueeze(2).broadcast_to([128, SK, 7]),
                    thr[:, 10:17].unsqueeze(1).broadcast_to([128, SK, 7]),
                    op=OP.is_ge)
                oh_r = maskp.tile([128, SK, 8], BF16, tag="oh_r")
                nc.vector.tensor_tensor(oh_r[:], mm[:, :, 0:8], mm[:, :, 1:9], op=OP.subtract)

                mt = maskp.tile([128, SK, 9], BF16, tag="mbuf")
                nc.vector.memset(mt[:, :, 0], 1.0)
                nc.vector.memset(mt[:, :, 8], 0.0)
                prodt = maskp.tile([128, SK, 7], F32, tag="prodt")
                nc.vector.tensor_tensor(
                    prodt[:],
                    rhoe[:].unsqueeze(2).broadcast_to([128, SK, 7]),
                    thr[:, 17:24].unsqueeze(1).broadcast_to([128, SK, 7]),
                    op=OP.mult)
                nc.vector.tensor_tensor(
                    mt[:, :, 1:8], prodt[:],
                    zs_.unsqueeze(2).broadcast_to([128, SK, 7]),
                    op=OP.is_gt)
                oh_t = maskp.tile([128, SK, 8], BF16, tag="oh_t")
                nc.vector.tensor_tensor(oh_t[:], mt[:, :, 0:8], mt[:, :, 1:9], op=OP.subtract)

                am = maskp.tile([128, SK], BF16, tag="am")
                bm = maskp.tile([128, SK], BF16, tag="bm")
                cm = maskp.tile([128, SK], BF16, tag="cm")
                ax_ = maskp.tile([128, SK], F32, tag="ax")
                ay_ = maskp.tile([128, SK], F32, tag="ay")
                nc.gpsimd.tensor_scalar(am[:], ys_, 0.0, None, op0=OP.is_ge)
                nc.gpsimd.tensor_scalar(bm[:], xs_, 0.0, None, op0=OP.is_ge)
                nc.scalar.activation(ax_[:], xs_, AF.Abs)
                nc.scalar.activation(ay_[:], ys_, AF.Abs)
                nc.vector.tensor_tensor(cm[:], ay_[:], ax_[:], op=OP.is_ge)
                na = maskp.tile([128, SK], BF16, tag="na")
                nb = maskp.tile([128, SK], BF16, tag="nb")
                ncc = maskp.tile([128, SK], BF16, tag="ncc")
                nc.gpsimd.tensor_scalar(na[:], am[:], -1.0, 1.0, op0=OP.mult, op1=OP.add)
                nc.gpsimd.tensor_scalar(nb[:], bm[:], -1.0, 1.0, op0=OP.mult, op1=OP.add)
                nc.gpsimd.tensor_scalar(ncc[:], cm[:], -1.0, 1.0, op0=OP.mult, op1=OP.add)
                t00 = maskp.tile([128, SK], BF16, tag="t00")
                t01 = maskp.tile([128, SK], BF16, tag="t01")
                t11 = maskp.tile([128, SK], BF16, tag="t11")
                t10 = maskp.tile([128, SK], BF16, tag="t10")
                nc.gpsimd.tensor_tensor(t00[:], na[:], nb[:], op=OP.mult)
                nc.gpsimd.tensor_tensor(t01[:], na[:], bm[:], op=OP.mult)
                nc.gpsimd.tensor_tensor(t11[:], am[:], bm[:], op=OP.mult)
                nc.gpsimd.tensor_tensor(t10[:], am[:], nb[:], op=OP.mult)
                oh_p = maskp.tile([128, SK, 8], BF16, tag="oh_p")
                for col, (t_, c_) in enumerate(
                        [(t00, ncc), (t00, cm), (t01, cm), (t01, ncc),
                         (t11, ncc), (t11, cm), (t10, cm), (t10, ncc)]):
                    nc.gpsimd.tensor_tensor(oh_p[:, :, col], t_[:], c_[:], op=OP.mult)

                sums = statp.tile([64, S, 4, 8], F32, tag="sums")
                for s in range(S):
                    sl = slice(s * NCH, (s + 1) * NCH)
                    eng = nc.vector if s < 10 else nc.gpsimd
                    ohrt = sampp.tile([128, NCH, 8, 8], BF16, tag="ohrt", name=f"ohrt{s}")
                    eng.tensor_tensor(
                        ohrt[:],
                        oh_r[:, sl, :].unsqueeze(3).broadcast_to([128, NCH, 8, 8]),
                        oh_t[:, sl, :].unsqueeze(2).broadcast_to([128, NCH, 8, 8]),
                        op=OP.mult)
                    whl = sampp.tile([128, NCH, 4, 8], BF16, tag="whl", name=f"whl{s}")
                    eng.tensor_tensor(
                        whl[:],
                        rel[:, :, s, :].rearrange("p c k -> p k c").unsqueeze(3)
                        .broadcast_to([128, NCH, 4, 8]),
                        oh_p[:, sl, :].unsqueeze(2).broadcast_to([128, NCH, 4, 8]),
                        op=OP.mult)

                    ps = psb.tile([64, 32], F32, tag="binp", name=f"binps{s}")
                    for k in range(NCH):
                        nc.tensor.matmul(ps[:], ohrt[:, k], whl[:, k],
                                         start=(k == 0), stop=(k == NCH - 1))
                    nc.scalar.activation(
                        sums[:, s, :, :].rearrange("p e f -> p (e f)"), ps[:],
                        AF.Copy)

                cnt = sums[:, :, 3, :]
                nc.vector.tensor_scalar(cnt, cnt, 1.0, None, op0=OP.max)
                nc.vector.reciprocal(cnt, cnt)
                binfb = statp.tile([64, S, 3, 8], BF16, tag="binfb")
                nc.vector.tensor_tensor(
                    binfb[:], sums[:, :, 0:3, :],
                    sums[:, :, 3:4, :].broadcast_to([64, S, 3, 8]), op=OP.mult)

                binf_d = dramp.tile([3, S * 512], BF16, tag="binfd")
                for c in range(3):
                    nc.sync.dma_start(
                        binf_d[c].rearrange("(s rt p) -> rt s p", s=S, rt=64, p=8),
                        binfb[:, :, c, :])

                # pad1 zero-fill split across Pool+DVE right after their
                # binning work; pad2/3 are emitted later, running while Pool
                # idles in conv phases
                zero1 = statp.tile([128, 1], BF16, tag="zero1")
                nc.vector.memset(zero1[:], 0.0)
                nc.gpsimd.memset(pad1[:, 0:PADW // 2], 0.0)

                epst = statp.tile([128, 1], F32, tag="epst")
                nc.vector.memset(epst[:], EPS)

                # ---------------- BN0 via binf moments ----------------
                # x = W_r @ binf is linear, so BN0 stats derive from binf's
                # first/second moments (12 numbers): the collective overlaps
                # the raise matmuls instead of stalling after them.
                mom = statp.tile([64, 9], F32, tag="mom")
                for c in range(3):
                    nc.vector.tensor_reduce(
                        mom[:, c:c + 1], binfb[:, :, c, :],
                        axis=AX.XY, op=OP.add)
                pairs = [(0, 0), (0, 1), (0, 2), (1, 1), (1, 2), (2, 2)]
                for j, (c1, c2) in enumerate(pairs):
                    mp = drainp.tile([64, S, 8], F32, tag="mprod", name=f"mp{j}")
                    nc.vector.tensor_tensor(mp[:], binfb[:, :, c1, :],
                                            binfb[:, :, c2, :], op=OP.mult)
                    nc.vector.tensor_reduce(
                        mom[:, 3 + j:4 + j],
                        mp[:].rearrange("p s f -> p (s f)"),
                        axis=AX.X, op=OP.add)
                ones64 = statp.tile([64, 1], F32, tag="ones64")
                nc.vector.memset(ones64[:], 1.0)
                mps = psb.tile([64, 32], F32, tag="binp", name="momp")
                nc.tensor.matmul(mps[0:1, 0:9], ones64[:], mom[:],
                                 start=True, stop=True)
                m9 = statp.tile([1, 9], F32, tag="m9")
                nc.scalar.activation(m9[:], mps[0:1, 0:9], AF.Copy)
                nc.scalar.activation(
                    pad1[:, PADW // 2:],
                    zero1[:].broadcast_to([128, PADW - PADW // 2]), AF.Copy)
                nc.scalar.activation(
                    pad2[:], zero1[:].broadcast_to([128, PADW]), AF.Copy)
                cci0 = dramp.tile([1, 9], F32, tag="cci0")
                cco0 = dramp.tile([n_cores, 9], F32, tag="cco0")
                # cci0 issues from the Act queue (right after m9) so it is
                # not stuck behind bulk SP transfers on the DMA device;
                # xr_all issues from the Pool queue after the collective, so
                # its 5us transfer runs during (not before) the collective.
                nc.scalar.dma_start(cci0[:], m9[:])
                if n_cores > 1:
                    nc.gpsimd.collective_compute(
                        "AllGather", OP.bypass,
                        replica_groups=[list(range(n_cores))],
                        ins=[cci0.opt()], outs=[cco0.opt()])
                else:
                    nc.sync.dma_start(cco0[:].rearrange("r x -> (r x)"),
                                      cci0[:].rearrange("p j -> (p j)"))
                xr_all = statp.tile([3, S * 512], BF16, tag="xrall")
                for q in range(4):
                    qs = q * 4 * 512
                    nc.sync.dma_start(xr_all[:, qs:qs + 4 * 512],
                                      binf_d[:, qs:qs + 4 * 512])
                gmb2 = statp.tile([128, n_cores, 9], F32, tag="gmb2")
                nc.sync.dma_start(
                    gmb2[:],
                    cco0[:].rearrange("r j -> (r j)").unsqueeze(0)
                    .partition_broadcast(128))
                gmb = statp.tile([128, 9], F32, tag="gmb")
                for j in range(9):
                    nc.vector.tensor_reduce(
                        gmb[:, j:j + 1], gmb2[:, :, j], axis=AX.X, op=OP.add)
                # mean = nstat * <Wrow, m>;  Ex2 = nstat * <qw, m2>
                mtmp = statp.tile([128, 9], F32, tag="mtmp")
                nc.vector.tensor_tensor(mtmp[:, 0:3], gmb[:, 0:3],
                                        gb[:, 24:27], op=OP.mult)
                nc.vector.tensor_tensor(mtmp[:, 3:9], gmb[:, 3:9],
                                        gb[:, 27:33], op=OP.mult)
                mean0 = statp.tile([128, 1], F32, tag="mean0")
                var0 = statp.tile([128, 1], F32, tag="var0")
                sc0 = statp.tile([128, 1], F32, tag="sc0")
                bi0 = statp.tile([128, 1], F32, tag="bi0")
                nc.vector.tensor_reduce(mean0[:], mtmp[:, 0:3], axis=AX.X, op=OP.add)
                nc.vector.tensor_scalar(mean0[:], mean0[:], nstat, None, op0=OP.mult)
                nc.vector.tensor_reduce(var0[:], mtmp[:, 3:9], axis=AX.X, op=OP.add)
                nc.vector.tensor_scalar(var0[:], var0[:], nstat, None, op0=OP.mult)
                nc.vector.tensor_tensor(mtmp[:, 0:1], mean0[:], mean0[:], op=OP.mult)
                nc.vector.tensor_tensor(var0[:], var0[:], mtmp[:, 0:1], op=OP.subtract)
                nc.scalar.activation(var0[:], var0[:], AF.Sqrt, bias=epst[:])
                nc.vector.reciprocal(var0[:], var0[:])
                nc.vector.tensor_tensor(sc0[:], gb[:, 0:1], var0[:], op=OP.mult)
                nc.vector.tensor_tensor(bi0[:], mean0[:], sc0[:], op=OP.mult)
                nc.vector.tensor_tensor(bi0[:], gb[:, 1:2], bi0[:], op=OP.subtract)

                # ---------------- raising (BN0+ReLU fused in drain) --------
                # The 32 raise channels are written as 3 w-shifted copies in
                # pad1's partition groups (copy g interior at w-offset g), so
                # conv1 can contract 3 taps per matmul (9 reads vs 27).
                # drains write RAW values and the w-shift copies are made
                # immediately — all hidden under the BN0 collective. The BN+
                # ReLU is applied per copy afterwards on three engines.
                CH = S // 2
                for c0 in range(0, S, CH):
                    for s in range(c0, c0 + CH):
                        xps = psg.tile([128, 512], F32, tag="big", name=f"xps{s}")
                        nc.tensor.matmul(
                            xps[:], wr4[:],
                            xr_all[:, s * 512:(s + 1) * 512],
                            start=True, stop=True)
                        v = box(pad1, s, 1, 1, 1, p0=32, p1=64)
                        nc.scalar.activation(
                            v, xps[32:64].rearrange("p (d h w) -> p d h w",
                                                    d=8, h=8), AF.Copy)
                    a, b = c0 * PD, (c0 + CH) * PD
                    nc.sync.dma_start(pad1[0:32, a:b - 1], pad1[32:64, a + 1:b])
                    nc.sync.dma_start(pad1[64:96, a + 1:b], pad1[32:64, a:b - 1])
                for s in range(S):
                    e2 = "act" if s % 2 else ("pool" if s >= 12 else "dve")
                    for gi, (lo, hi, dw, eng) in enumerate(
                            ((32, 64, 1, "act"), (0, 32, 0, "dve"),
                             (64, 96, 2, e2))):
                        vg = box(pad1, s, 1, 1, dw, p0=lo, p1=hi)
                        if eng == "act":
                            nc.scalar.activation(vg, vg, AF.Relu,
                                                 bias=bi0[lo:hi], scale=sc0[lo:hi])
                        else:
                            e = nc.vector if eng == "dve" else nc.gpsimd
                            t0 = drainp.tile([32, 8, 8, 8], BF16, tag="rl0",
                                             name=f"rl0_{s}_{gi}")
                            e.tensor_scalar(t0[:], vg, sc0[lo:hi], bi0[lo:hi],
                                            op0=OP.mult, op1=OP.add)
                            e.tensor_scalar(vg, t0[:], 0.0, None, op0=OP.max)

                def bn_allreduce(st, layer, dup64=False):
                    # AllGather + local reduce: collective latency is ~1.9x
                    # smaller than AllReduce for these tiny payloads.
                    ncols = st.shape[-1]
                    cc_in = dramp.tile([128, ncols], F32, tag=f"cci{layer}")
                    cc_out = dramp.tile([n_cores, 128 * ncols], F32, tag=f"cco{layer}")
                    nc.sync.dma_start(cc_in[:], st[:])
                    if n_cores > 1:
                        nc.gpsimd.collective_compute(
                            "AllGather", OP.bypass,
                            replica_groups=[list(range(n_cores))],
                            ins=[cc_in.opt()], outs=[cc_out.opt()])
                    else:
                        nc.sync.dma_start(
                            cc_out[:].rearrange("r x -> (r x)"),
                            cc_in[:].rearrange("p j -> (p j)"))
                    grt = statp.tile([128, ncols, n_cores], F32, tag=f"grt{layer}")
                    src_ap = cc_out[:].rearrange("r (p j) -> p j r", p=128, j=ncols)
                    if dup64:
                        nc.sync.dma_start(grt[0:64], src_ap[0:64])
                        nc.sync.dma_start(grt[64:128], src_ap[0:64])
                    else:
                        nc.sync.dma_start(grt[:], src_ap)
                    gst = statp.tile([128, ncols], F32, tag=f"gst{layer}")
                    for j in range(ncols):
                        nc.vector.tensor_reduce(
                            gst[:, j:j + 1], grt[:, j, :], axis=AX.X, op=OP.add)
                    return gst

                def bn_math(gp, gq, gcol, bcol, layer, off=0):
                    mean = statp.tile([128, 1], F32, tag=f"mean{layer}_{off}")
                    var = statp.tile([128, 1], F32, tag=f"var{layer}_{off}")
                    sc = statp.tile([128, 1], F32, tag=f"sc{layer}_{off}")
                    bi = statp.tile([128, 1], F32, tag=f"bi{layer}_{off}")
                    nc.vector.tensor_scalar(mean[:], gp, nstat, None, op0=OP.mult)
                    nc.vector.tensor_tensor(var[:], mean[:], mean[:], op=OP.mult)
                    nc.vector.scalar_tensor_tensor(var[:], gq, nstat, var[:],
                                                   op0=OP.mult, op1=OP.subtract)
                    nc.scalar.activation(var[:], var[:], AF.Sqrt, bias=epst[:])
                    nc.vector.reciprocal(var[:], var[:])
                    nc.vector.tensor_tensor(sc[:], gb[:, gcol:gcol + 1], var[:], op=OP.mult)
                    nc.vector.tensor_tensor(bi[:], mean[:], sc[:], op=OP.mult)
                    nc.vector.tensor_tensor(bi[:], gb[:, bcol:bcol + 1], bi[:], op=OP.subtract)
                    return sc, bi

                def bn_block(ptile, qtile, gcol, bcol, layer, dup64=False):
                    st = statp.tile([128, 2], F32, tag=f"st{layer}")
                    nc.vector.tensor_reduce(st[:, 0:1], ptile[:], axis=AX.X, op=OP.add)
                    nc.vector.tensor_reduce(st[:, 1:2], qtile[:], axis=AX.X, op=OP.add)
                    gst = bn_allreduce(st, layer, dup64)
                    return bn_math(gst[:, 0:1], gst[:, 1:2], gcol, bcol, layer)

                # conv1: 9 reads/sample, each contracting 3 w-taps across the
                # shifted pad1 copies. Output 64ch written as 2 w-shifted
                # copies in pad2 so conv2 can contract 2 taps per matmul.
                pt1 = statp.tile([128, S], F32, tag="pt1")
                qt1 = statp.tile([128, S], F32, tag="qt1")
                nc.vector.memset(pt1[:], 0.0)
                nc.vector.memset(qt1[:], 0.0)
                for s0 in range(0, S, 4):
                    pss = [psg.tile([128, 512], F32, tag="big", name=f"c1ps_{s0}_{g}")
                           for g in range(4)]
                    for r in range(9):
                        a, bb = r // 3, r % 3
                        for g in range(4):
                            nc.tensor.matmul(pss[g][:], w1p[:, r * 128:(r + 1) * 128],
                                             box(pad1, s0 + g, a, bb, 1),
                                             start=(r == 0), stop=(r == 8))
                    for g in range(4):
                        s = s0 + g
                        v = box(pad2, s, 1, 1, 0, p0=0, p1=64)
                        nc.scalar.activation(
                            v, pss[g][0:64].rearrange("p (d h w) -> p d h w", d=8, h=8),
                            AF.Copy, accum_out=pt1[0:64, s:s + 1])
                        sq = drainp.tile([64, 8, 8, 8], BF16, tag="sqc1", name=f"sqc1_{s0}_{g}")
                        nc.vector.tensor_tensor(sq[:], v, v, op=OP.mult)
                        nc.vector.tensor_reduce(
                            qt1[0:64, s:s + 1],
                            sq[:].rearrange("p d h w -> p (d h w)"),
                            axis=AX.X, op=OP.add)
                    # replicate the raw group into copy 1 now (overlaps conv1
                    # and the BN1 collective); relu is applied per copy later.
                    a, b = s0 * PD, (s0 + 4) * PD
                    nc.sync.dma_start(pad2[64:128, a + 1:b], pad2[0:64, a:b - 1])
                sc1, bi1 = bn_block(pt1, qt1, 2, 3, 1, dup64=True)
                for s in range(S):
                    v = box(pad2, s, 1, 1, 0, p0=0, p1=64)
                    nc.scalar.activation(v, v, AF.Relu,
                                         bias=bi1[0:64], scale=sc1[0:64])
                    v2 = box(pad2, s, 1, 1, 1, p0=64, p1=128)
                    if s % 4 == 3:
                        nc.scalar.activation(v2, v2, AF.Relu,
                                             bias=bi1[64:128], scale=sc1[64:128])
                    else:
                        t2 = drainp.tile([64, 8, 8, 8], BF16, tag="rl1", name=f"rl1_{s}")
                        nc.vector.tensor_scalar(t2[:], v2, sc1[64:128], bi1[64:128],
                                                op0=OP.mult, op1=OP.add)
                        nc.vector.tensor_scalar(v2, t2[:], 0.0, None, op0=OP.max)

                # conv2: 18 reads/sample (2 per (d,h) tap pair), contracting
                # 2 w-taps across the shifted pad2 copies.
                nc.gpsimd.memset(pad3[:], 0.0)
                pt2 = statp.tile([128, S], F32, tag="pt2")
                qt2 = statp.tile([128, S], F32, tag="qt2")
                for s0 in range(0, S, 4):
                    pss = [psg.tile([128, 512], F32, tag="big", name=f"c2ps_{s0}_{g}")
                           for g in range(4)]
                    for r in range(18):
                        a, bb, tw = r // 6, (r // 2) % 3, r % 2
                        for g in range(4):
                            nc.tensor.matmul(pss[g][:], w2p[:, r * 128:(r + 1) * 128],
                                             box(pad2, s0 + g, a, bb, tw),
                                             start=(r == 0), stop=(r == 17))
                    for g in range(4):
                        s = s0 + g
                        v = box(pad3, s, 1, 1, 1)
                        nc.scalar.activation(
                            v, pss[g][:].rearrange("p (d h w) -> p d h w", d=8, h=8),
                            AF.Copy, accum_out=pt2[:, s:s + 1])
                        sq = drainp.tile([128, 8, 8, 8], BF16, tag="sq", name=f"sqc2_{s0}_{g}")
                        nc.vector.tensor_tensor(sq[:], v, v, op=OP.mult)
                        nc.vector.tensor_reduce(
                            qt2[:, s:s + 1],
                            sq[:].rearrange("p d h w -> p (d h w)"),
                            axis=AX.X, op=OP.add)
                sc2, bi2 = bn_block(pt2, qt2, 4, 5, 2)
                for s in range(S):
                    v = box(pad3, s, 1, 1, 1)
                    if s % 2 == 0:
                        nc.scalar.activation(v, v, AF.Relu, bias=bi2[:], scale=sc2[:])
                    else:
                        t2 = drainp.tile([128, 8, 8, 8], BF16, tag="rl2", name=f"rl2_{s}")
                        nc.vector.tensor_scalar(t2[:], v, sc2[:], bi2[:],
                                                op0=OP.mult, op1=OP.add)
                        nc.vector.tensor_scalar(v, t2[:], 0.0, None, op0=OP.max)

                # h-outer: half 0's BN3 collective overlaps half 1's convs
                desc_sb = statp.tile([128, 2, S], F32, tag="descsb")
                for h, (gcol, bcol) in ((0, (6, 7)), (1, (8, 9))):
                    pt3 = statp.tile([128, S], F32, tag=f"pt3_{h}")
                    qt3 = statp.tile([128, S], F32, tag=f"qt3_{h}")
                    mxs = statp.tile([128, S], F32, tag=f"mxs_{h}")
                    mns = statp.tile([128, S], F32, tag=f"mns_{h}")
                    for s in range(S):
                        ps = psg.tile([128, 512], F32, tag="big", name=f"c3ps_{s}_{h}")
                        for o in range(27):
                            dd, dh, dw = o // 9, (o // 3) % 3, o % 3
                            lhsT = w3p[:, (o * 2 + h) * 128:(o * 2 + h + 1) * 128]
                            rhs = box(pad3, s, dd, dh, dw)
                            nc.tensor.matmul(ps[:], lhsT, rhs, start=(o == 0), stop=(o == 26))
                        nc.vector.tensor_reduce(mxs[:, s:s + 1], ps[:], axis=AX.X, op=OP.max)
                        nc.vector.tensor_reduce(mns[:, s:s + 1], ps[:], axis=AX.X, op=OP.min)
                        nc.vector.tensor_reduce(pt3[:, s:s + 1], ps[:], axis=AX.X, op=OP.add)
                        sq = drainp.tile([128, 512], BF16, tag="sqf", name=f"sqf_{s}_{h}")
                        nc.scalar.activation(sq[:], ps[:], AF.Square)
                        nc.vector.tensor_reduce(qt3[:, s:s + 1], sq[:], axis=AX.X, op=OP.add)

                    sc3, bi3 = bn_block(pt3, qt3, gcol, bcol, 3 + h)
                    zmx = statp.tile([128, S], F32, tag=f"zmx{h}")
                    zmn = statp.tile([128, S], F32, tag=f"zmn{h}")
                    nc.vector.tensor_scalar(zmx[:], mxs[:], sc3[:], bi3[:],
                                            op0=OP.mult, op1=OP.add)
                    nc.vector.tensor_scalar(zmn[:], mns[:], sc3[:], bi3[:],
                                            op0=OP.mult, op1=OP.add)
                    csel = statp.tile([128, 1], F32, tag=f"csel{h}")
                    nc.vector.tensor_scalar(csel[:], sc3[:], 0.0, None, op0=OP.is_ge)
                    nc.vector.tensor_tensor(zmx[:], zmx[:], zmn[:], op=OP.subtract)
                    nc.vector.scalar_tensor_tensor(zmx[:], zmx[:], csel[:], zmn[:],
                                                   op0=OP.mult, op1=OP.add)
                    nc.vector.tensor_scalar(desc_sb[:, h, :], zmx[:], 0.0, None, op0=OP.max)
                    nc.sync.dma_start(
                        desc_d.ap().rearrange("s (hh ch) -> ch hh s", hh=2)[:, h, :],
                        desc_sb[:, h, :])

                if extra_cc:
                    # calibration chain: extra_cc serial AllGathers, each
                    # reading row 0 of the previous gather's output
                    xa = dramp.tile([n_cores, 16], F32, tag="xcc_a")
                    xb = dramp.tile([n_cores, 16], F32, tag="xcc_b")
                    nc.sync.dma_start(
                        xa[:].rearrange("r x -> (r x)")[0:16],
                        desc_sb[0:1, 0, 0:16].rearrange("p a -> (p a)"))
                    bufs_cc = [xa, xb]
                    for i in range(extra_cc):
                        src_t, dst_t = bufs_cc[i % 2], bufs_cc[(i + 1) % 2]
                        nc.gpsimd.collective_compute(
                            "AllGather", OP.bypass,
                            replica_groups=[list(range(n_cores))],
                            ins=[src_t[0:1, :]], outs=[dst_t.opt()])

    nc.compile()
    return nc


def _host_pack(inputs):
    pts_all = np.asarray(inputs["points"], np.float32)
    ctr_all = np.asarray(inputs["center_points"], np.float32)
    w_raise = np.asarray(inputs["w_raise"], np.float32)
    w1 = np.asarray(inputs["w1"], np.float32)
    w2 = np.asarray(inputs["w2"], np.float32)
    w3 = np.asarray(inputs["w3"], np.float32)

    wr4 = np.zeros((3, 128), np.float32)
    for g in range(4):
        wr4[:, g * 32:(g + 1) * 32] = w_raise.T
    # conv1: 9 reads at (a, b, 1); pad1 copy g (partitions 32g:32g+32) is
    # written at w-offset g, so at read (a, b, 1) it contributes tap
    # (a, b, 2-g). Copy 3 (partitions 96:128) is unused (zero weights/data).
    w1f = w1.reshape(64, 32, 27)
    w1p = np.zeros((128, 9 * 128), np.float32)
    for a in range(3):
        for bb in range(3):
            r = a * 3 + bb
            for g in range(3):
                o = a * 9 + bb * 3 + (2 - g)
                blk = np.concatenate([w1f[:, :, o].T, w1f[:, :, o].T], axis=1)
                w1p[32 * g:32 * (g + 1), r * 128:(r + 1) * 128] = blk
    # conv2: 2 reads per (a, b): read (a, b, 0) gives taps w=1 (copy 0,
    # written at w-offset 0) and w=0 (copy 1, at w-offset 1); read (a, b, 1)
    # gives tap w=2 via copy 0 (copy 1 rows zeroed to avoid double-count).
    w2f = w2.reshape(128, 64, 27)
    w2p = np.zeros((128, 18 * 128), np.float32)
    for a in range(3):
        for bb in range(3):
            r2 = (a * 3 + bb) * 2
            w2p[0:64, r2 * 128:(r2 + 1) * 128] = w2f[:, :, a * 9 + bb * 3 + 1].T
            w2p[64:128, r2 * 128:(r2 + 1) * 128] = w2f[:, :, a * 9 + bb * 3 + 0].T
            w2p[0:64, (r2 + 1) * 128:(r2 + 2) * 128] = w2f[:, :, a * 9 + bb * 3 + 2].T
    w3f = w3.reshape(256, 128, 27)
    w3p = np.zeros((128, 54 * 128), np.float32)
    for o in range(27):
        for h in range(2):
            w3p[:, (o * 2 + h) * 128:(o * 2 + h + 1) * 128] = \
                w3f[h * 128:(h + 1) * 128, :, o].T
    gb = np.zeros((128, 36), np.float32)
    g0 = np.asarray(inputs["g0"], np.float32); be0 = np.asarray(inputs["be0"], np.float32)
    g1 = np.asarray(inputs["g1"], np.float32); be1 = np.asarray(inputs["be1"], np.float32)
    g2 = np.asarray(inputs["g2"], np.float32); be2 = np.asarray(inputs["be2"], np.float32)
    g3 = np.asarray(inputs["g3"], np.float32); be3 = np.asarray(inputs["be3"], np.float32)
    gb[:, 0] = np.tile(g0, 4); gb[:, 1] = np.tile(be0, 4)
    gb[:, 2] = np.tile(g1, 2); gb[:, 3] = np.tile(be1, 2)
    gb[:, 4] = g2; gb[:, 5] = be2
    gb[:, 6] = g3[:128]; gb[:, 7] = be3[:128]
    gb[:, 8] = g3[128:]; gb[:, 9] = be3[128:]
    gb[:, 10:17] = np.asarray(R2_B, np.float32)[None, :]
    gb[:, 17:24] = np.asarray(COS_B, np.float32)[None, :]
    wr_rows = np.tile(w_raise, (4, 1))                    # [128, 3]
    gb[:, 24:27] = wr_rows
    qw = np.stack([wr_rows[:, 0] ** 2, 2 * wr_rows[:, 0] * wr_rows[:, 1],
                   2 * wr_rows[:, 0] * wr_rows[:, 2], wr_rows[:, 1] ** 2,
                   2 * wr_rows[:, 1] * wr_rows[:, 2], wr_rows[:, 2] ** 2], axis=1)
    gb[:, 27:33] = qw                                     # [128, 6]

    shared = dict(
        wr4=wr4.astype(ml_dtypes.bfloat16),
        w1p=w1p.astype(ml_dtypes.bfloat16),
        w2p=w2p.astype(ml_dtypes.bfloat16),
        w3p=w3p.astype(ml_dtypes.bfloat16),
        gb=gb,
    )
    in_maps = []
    for c in range(NCORES):
        b = c // 4
        pts = np.ascontiguousarray(np.transpose(pts_all[b].reshape(NCH, 128, 3), (1, 0, 2)))
        s0 = (c % 4) * S
        ctrv = np.ascontiguousarray(ctr_all[b, s0:s0 + S, :].reshape(-1))
        in_maps.append(dict(pts=pts, ctrv=ctrv, **shared))
    return in_maps


_CACHED_NC = None


def _get_nc():
    global _CACHED_NC
    if _CACHED_NC is None:
        _CACHED_NC = _build_nc(NCORES)
    return _CACHED_NC


class _Runner:
    """Persistent SPMD executor: jit once, keep stable inputs device-resident.

    Mirrors bass2jax.run_bass_via_pjrt's axon path, but the jitted
    executable and the (large, call-invariant) weight buffers live across
    calls, so each call ships only the small per-call tensors.
    """

    def __init__(self, nc, n_cores, static_names=(), chain=1):
        import jax
        import concourse.mybir as _mybir
        from jax.sharding import Mesh, PartitionSpec, NamedSharding
        from jax.experimental.shard_map import shard_map
        from concourse.bass2jax import (
            _bass_exec_p, partition_id_tensor, install_neuronx_cc_hook)

        install_neuronx_cc_hook()
        self.jax = jax
        self.nc = nc
        self.n_cores = n_cores
        self.static_names = set(static_names)
        self._static_cache = {}

        pname = nc.partition_id_tensor.name if nc.partition_id_tensor else None
        in_names, out_names, out_avals, zero_shapes = [], [], [], []
        for alloc in nc.m.functions[0].allocations:
            if not isinstance(alloc, _mybir.MemoryLocationSet):
                continue
            name = alloc.memorylocations[0].name
            if alloc.kind == "ExternalInput":
                if name != pname:
                    in_names.append(name)
            elif alloc.kind == "ExternalOutput":
                out_names.append(name)
                shape = tuple(alloc.tensor_shape)
                dtype = _mybir.dt.np(alloc.dtype)
                out_avals.append(jax.core.ShapedArray(shape, dtype))
                zero_shapes.append((shape, dtype))
        self.in_names, self.out_names = in_names, out_names
        self.out_avals, self.zero_shapes = out_avals, zero_shapes
        n_params, n_outs = len(in_names), len(out_names)
        in_names_full = in_names + out_names + ([pname] if pname else [])

        def _body(*args):
            ins, zeros = list(args[:n_params]), list(args[n_params:])
            pid = [partition_id_tensor()] if pname is not None else []
            for _ in range(chain):
                zeros = list(_bass_exec_p.bind(
                    *ins, *zeros, *pid, out_avals=tuple(out_avals),
                    in_names=tuple(in_names_full), out_names=tuple(out_names),
                    lowering_input_output_aliases=(), sim_require_finite=True,
                    sim_require_nnan=True, nc=nc))
            return tuple(zeros)

        devices = jax.devices()[:n_cores]
        self.mesh = Mesh(np.array(devices), ("core",))
        self.sharding = NamedSharding(self.mesh, PartitionSpec("core"))
        in_specs = (PartitionSpec("core"),) * (n_params + n_outs)
        out_specs = (PartitionSpec("core"),) * n_outs
        donate = tuple(range(n_params, n_params + n_outs))
        self.sharded = jax.jit(
            shard_map(_body, mesh=self.mesh, in_specs=in_specs,
                      out_specs=out_specs, check_rep=False),
            donate_argnums=donate, keep_unused=True)

    def __call__(self, in_maps):
        jax = self.jax
        args = []
        for i, name in enumerate(self.in_names):
            if name in self.static_names and name in self._static_cache:
                args.append(self._static_cache[name])
                continue
            cat = np.concatenate(
                [np.asarray(in_maps[c][name]) for c in range(self.n_cores)],
                axis=0)
            arr = jax.device_put(cat, self.sharding)
            if name in self.static_names:
                self._static_cache[name] = arr
            args.append(arr)
        for shape, dtype in self.zero_shapes:
            args.append(np.zeros((self.n_cores * shape[0], *shape[1:]), dtype))
        outs = self.sharded(*args)
        return [
            {name: np.asarray(outs[i]).reshape(self.n_cores, *self.out_avals[i].shape)[c]
             for i, name in enumerate(self.out_names)}
            for c in range(self.n_cores)
        ]


_RUNNER = None
_WKEY = None

_STATIC_INPUTS = ("wr4", "w1p", "w2p", "w3p", "gb")


def _weights_key(in_maps):
    import hashlib
    h = hashlib.sha1()
    for name in _STATIC_INPUTS:
        h.update(np.ascontiguousarray(in_maps[0][name]).tobytes())
    return h.digest()


def kernel(**inputs) -> np.ndarray:
    global _RUNNER, _WKEY
    nc = _get_nc()
    in_maps = _host_pack(inputs)
    try:
        if _RUNNER is None:
            _RUNNER = _Runner(nc, NCORES, static_names=_STATIC_INPUTS)
        wkey = _weights_key(in_maps)
        if wkey != _WKEY:
            _RUNNER._static_cache.clear()
            _WKEY = wkey
        results = _RUNNER(in_maps)
    except Exception:
        res = bass_utils.run_bass_kernel_spmd(
            nc, in_maps, core_ids=list(range(NCORES)))
        results = res.results
    out = np.concatenate([np.asarray(results[c]["desc"], np.float32)
                          for c in range(NCORES)], axis=0)     # (128, 256)
    return out.reshape(B, M, 256)

